# revision 1
# baseline (speedup 1.0000x reference)
"""Trainium2 Bass kernel for nn_Aggregate (GNN message passing / COO SpMM + Linear).

Computes: y = segment_sum(edge_val[:,None] * x[edge_col], edge_row, N) @ W.T

Strategy (8 NeuronCores, SPMD):
  - Shard destination nodes across the 8 cores (N/8 rows each); sort edges by
    destination on the host and route each edge to the core owning its dest row.
  - Replicate x (fp16) in every core's HBM. Each core gathers x[edge_col] rows
    for its edges with gpsimd dma_gather (int16 indices -> x is split into row
    banks of <=32k rows; edges are grouped by (dest window, bank) with each
    group padded to a multiple of 128).
  - The one-hot selection matrices sel[e, r] = (r == row_local[e]) * val[e]
    are PRECOMPUTED ON THE HOST (they depend only on indices/vals, not x) and
    streamed from HBM as fp16 -- big contiguous DMA at full bandwidth, zero
    compute-engine cost on device.
  - Segment-sum on the TensorEngine, transposed: accumulate
    yT[d, r] += sum_e G[e, d] * sel[e, r] in PSUM (lhsT=G chunk, rhs=sel).
  - Per window: out = yT.T @ W.T via one matmul (lhsT=yT_sbuf, rhs=W.T) -- no
    PE transpose; PSUM<->SBUF copies run on the scalar engine.

Self-contained: numpy + the concourse/bass stack at /opt/trn_rl_repo.
"""

import os
import sys

for _p in ("/opt/trn_rl_repo",):
    if _p not in sys.path and os.path.isdir(_p):
        sys.path.insert(0, _p)

import numpy as np

import concourse.bass as bass
import concourse.mybir as mybir
import concourse.tile as tile
from concourse import bacc
from concourse.bass_utils import run_bass_kernel_spmd

P = 128
NCORES = 8
MAX_BANK = 32000  # int16 index headroom
F32 = mybir.dt.float32
F16 = mybir.dt.float16
I16 = mybir.dt.int16

# Populated by the most recent kernel() call (test harness reads these).
LAST_RESULTS = None


def _install_ntff_shim():
    """The agent image's `antenv` lacks `axon_hooks`; provide it so
    run_bass_kernel_spmd(trace=True) can reach the NTFF profiler."""
    import types

    if "antenv.axon_hooks" in sys.modules:
        return
    try:
        from trn_agent_boot.trn_boot import _ntff_profile_via_ctypes
    except ImportError:
        return
    hook = _ntff_profile_via_ctypes("/opt/axon/libaxon_pjrt.so")
    mod = types.ModuleType("antenv.axon_hooks")
    mod.get_axon_ntff_profile_hook = lambda: hook
    mod.set_axon_ntff_profile_hook = lambda h: None
    sys.modules["antenv.axon_hooks"] = mod
    # the artifact upload wants a remote bucket that is unreachable here
    import concourse.bass_utils as _bu

    _bu.upload_artifacts = lambda tmpdir: f"local:{tmpdir}"


def _balance_permutation(row, col, n_nodes, n_banks, bank_size):
    """Permute dest-node ids so per-(window, bank) edge counts pack just under
    multiples of 128 and stay balanced across cores. Shrinks the shared chunk
    count (pure padding -> pure perf; correctness is permutation-independent).
    Returns perm[n_nodes]: node -> new id."""
    NB = n_banks
    shard = n_nodes // NCORES
    n_win = (shard + P - 1) // P
    last_size = shard - (n_win - 1) * P
    n_full = (n_win - 1) * NCORES
    n_windows = n_win * NCORES

    bank_of = col // bank_size
    deg = np.bincount(row * NB + bank_of, minlength=n_nodes * NB).reshape(
        n_nodes, NB
    ).astype(np.int64)

    caps = np.full(n_windows, P, dtype=np.int64)
    caps[n_full:] = last_size

    lam = len(row) / n_windows / max(NB, 1)
    T = int(max(np.floor(lam / P + 1) * P - 6, 1))

    tall = np.zeros((n_windows, NB), dtype=np.int64)
    size = np.zeros(n_windows, dtype=np.int64)
    assign = np.full(n_nodes, -1, dtype=np.int64)

    order = np.argsort(-deg.sum(1), kind="stable")
    nb_pen = max(NB - 1, 1)
    degs = deg[order]
    FULL = 1 << 40
    for i in range(len(order)):
        d = degs[i]
        over_new = tall[:, :nb_pen] + d[None, :nb_pen] - T
        np.maximum(over_new, 0, out=over_new)
        over_old = tall[:, :nb_pen] - T
        np.maximum(over_old, 0, out=over_old)
        cost = (over_new * over_new - over_old * over_old).sum(1)
        cost += np.where(size >= caps, FULL, 0)
        w = int(np.argmin(cost))
        assign[order[i]] = w
        tall[w] += d
        size[w] += 1

    # group similar windows into grid rows so max-over-cores stays tight
    chunk_demand = (tall + P - 1) // P
    full_ids = np.arange(n_full)
    keys = [chunk_demand[full_ids, b] for b in range(NB)] + [tall[full_ids, NB - 1]]
    lex = np.lexsort(tuple(reversed(keys)))
    full_sorted = full_ids[lex]

    perm = np.empty(n_nodes, dtype=np.int64)
    order_in_win = np.argsort(assign, kind="stable")
    win_start = np.searchsorted(assign[order_in_win], np.arange(n_windows))
    win_end = np.concatenate([win_start[1:], [n_nodes]])

    for r in range(n_win - 1):
        for c in range(NCORES):
            w = int(full_sorted[r * NCORES + c])
            nodes = order_in_win[win_start[w] : win_end[w]]
            base = c * shard + r * P
            perm[nodes] = base + np.arange(len(nodes))
    for c in range(NCORES):
        w = n_full + c
        nodes = order_in_win[win_start[w] : win_end[w]]
        base = c * shard + (n_win - 1) * P
        perm[nodes] = base + np.arange(len(nodes))

    return perm


def _preprocess(edge_row, edge_col, edge_val, n_nodes):
    """Sort edges by dest, shard by dest across cores, group each core's
    window edges by source bank, pad each (window, bank) group to a multiple
    of 128 (shared counts across cores for SPMD).

    Returns:
      wrap_sb: [NCORES, P, max_icols] int16 (dma_gather index wrap, packed)
      sel: [NCORES, P, k_total*P] fp16 host-built selection matrices
           (sel[c, p, k*P + r] = val of edge at slot (p, k) if its local dest
            row is r, else 0)
      chunk_bank, chunk_slot: [K_total] int arrays: which bank stream + slot
        each global chunk reads from
      win_chunks: [n_win] number of chunks per window
      n_win, shard, n_banks, bank_size
    """
    shard = n_nodes // NCORES
    n_win = (shard + P - 1) // P
    n_banks = max(1, -(-n_nodes // MAX_BANK))
    bank_size = -(-n_nodes // n_banks)
    NB = n_banks

    row = np.asarray(edge_row).astype(np.int64).ravel()
    col = np.asarray(edge_col).astype(np.int64).ravel()
    val = np.asarray(edge_val).astype(np.float32).ravel()

    if os.environ.get("KBAL", "1") == "1" and n_nodes % NCORES == 0:
        perm = _balance_permutation(row, col, n_nodes, n_banks, bank_size)
        row = perm[row]
    else:
        perm = None

    core_of = row // shard
    win_of = (row - core_of * shard) // P
    row_local = (row - core_of * shard - win_of * P).astype(np.int64)
    bank_of = col // bank_size

    # group key per edge: (core, win, bank)
    key = (core_of * n_win + win_of) * NB + bank_of
    counts = np.bincount(key, minlength=NCORES * n_win * NB).reshape(
        NCORES, n_win, NB
    )
    # shared chunk counts: max over cores
    chunks_wb = (counts.max(axis=0) + P - 1) // P  # [n_win, NB]
    # ensure every window has at least one chunk
    empty = chunks_wb.sum(axis=1) == 0
    chunks_wb[empty, 0] = 1

    win_chunks = chunks_wb.sum(axis=1)  # [n_win]
    k_total = int(win_chunks.sum())

    # global chunk order: w-major, then bank
    flat_chunks = chunks_wb.ravel()  # [(w,b)] -> count
    grp_chunk_base = np.concatenate([[0], np.cumsum(flat_chunks)])[:-1]  # global
    # per-bank slot base for each (w,b) group
    bank_len = chunks_wb.sum(axis=0)  # [NB] chunks per bank stream
    grp_bank_base = np.zeros((n_win, NB), dtype=np.int64)
    grp_bank_base[1:] = np.cumsum(chunks_wb[:-1], axis=0)

    # chunk -> (bank, slot) mapping
    chunk_bank = np.zeros(k_total, dtype=np.int64)
    chunk_slot = np.zeros(k_total, dtype=np.int64)
    for w in range(n_win):
        for b in range(NB):
            n = chunks_wb[w, b]
            if n == 0:
                continue
            g0 = grp_chunk_base[w * NB + b]
            chunk_bank[g0 : g0 + n] = b
            chunk_slot[g0 : g0 + n] = grp_bank_base[w, b] + np.arange(n)

    wrap_idx = [
        np.zeros((NCORES, int(bank_len[b]) * P), dtype=np.int16) for b in range(NB)
    ]

    order = np.argsort(key, kind="stable")
    key_s = key[order]
    # position of each edge within its (core,win,bank) group
    grp_change = np.concatenate([[True], key_s[1:] != key_s[:-1]])
    grp_start = np.flatnonzero(grp_change)
    rep = np.diff(np.concatenate([grp_start, [key_s.shape[0]]]))
    pos_in_grp = np.arange(key_s.shape[0]) - np.repeat(grp_start, rep)

    core_s = key_s // (n_win * NB)
    wb_s = key_s % (n_win * NB)
    w_s = wb_s // NB
    b_s = wb_s % NB
    gchunk = grp_chunk_base[wb_s] + pos_in_grp // P  # global chunk id
    p_s = pos_in_grp % P
    bslot = grp_bank_base[w_s, b_s] + pos_in_grp // P  # bank-stream slot
    q_s = bslot * P + p_s  # bank-stream position

    col_rebased = (col[order] - b_s * bank_size).astype(np.int16)
    rl_s = row_local[order]
    v_s = val[order]

    # host-built selection matrices, fp16, partition-major for clean DMA:
    # sel[c, p, k*P + r]
    sel = np.zeros((NCORES, P, k_total * P), dtype=np.float16)
    sel[core_s, p_s, gchunk * P + rl_s] = v_s

    for c in range(NCORES):
        m = core_s == c
        for b in range(NB):
            mb = m & (b_s == b)
            wrap_idx[b][c, q_s[mb]] = col_rebased[mb]

    # packed wrap layout. Two modes:
    #  KQ=1 (default): ONE queue, each bank stream gets its own column range
    #    [bank_col_base[b], ...); idx wrapped in 16 partitions and replicated
    #    to all 128 partitions (8 Q7 cores each read their copy -> full-rate
    #    descriptor generation, benchmark configuration).
    #  KQ=4: legacy 4-queue banding -- bank b in partition band 32b..32b+31,
    #    replicated to the band's two 16-row groups.
    legacy_q = os.environ.get("KQ", "4") == "4"
    if legacy_q:
        max_cols = max(int(bank_len[b]) * P // 16 for b in range(NB))
        bank_col_base = [0] * NB
        wrap_sb = np.zeros((NCORES, P, max_cols), dtype=np.int16)
        for b in range(NB):
            lb = int(bank_len[b]) * P
            if lb == 0:
                continue
            band = (b % 4) * 32
            for c in range(NCORES):
                blk = wrap_idx[b][c].reshape(lb // 16, 16).T  # [16, lb/16]
                wrap_sb[c, band : band + 16, : lb // 16] = blk
                wrap_sb[c, band + 16 : band + 32, : lb // 16] = blk
    else:
        bank_col_base = np.concatenate(
            [[0], np.cumsum([int(bank_len[b]) * P // 16 for b in range(NB)])]
        )[:-1].tolist()
        max_cols = sum(int(bank_len[b]) * P // 16 for b in range(NB))
        wrap_sb = np.zeros((NCORES, P, max_cols), dtype=np.int16)
        for b in range(NB):
            lb = int(bank_len[b]) * P
            if lb == 0:
                continue
            c0 = bank_col_base[b]
            for c in range(NCORES):
                blk = wrap_idx[b][c].reshape(lb // 16, 16).T  # [16, lb/16]
                wrap_sb[c, :, c0 : c0 + lb // 16] = np.tile(blk, (8, 1))

    return (
        wrap_sb,
        sel,
        chunk_bank,
        chunk_slot,
        bank_len.astype(np.int64),
        bank_col_base,
        win_chunks.astype(np.int64),
        n_win,
        shard,
        NB,
        bank_size,
        perm,
    )


def _build(n_nodes, k_total, win_chunks, chunk_bank, chunk_slot, bank_len,
           bank_col_base, n_banks, bank_size, gather_batch, sel_batch,
           max_icols):
    """Build the SPMD Bass program (same program on all 8 cores)."""
    n_win = len(win_chunks)
    legacy_q = os.environ.get("KQ", "4") == "4"
    n_queues = min(4, max(1, n_banks)) if legacy_q else 1
    single_packet = os.environ.get("KSP", "0") == "1"
    nc = bacc.Bacc("TRN2", target_bir_lowering=False, debug=False,
                   num_swdge_queues=n_queues,
                   dynamic_dma_scratch_size=int(os.environ.get("KSCRATCH", "32768")))

    x_d = nc.dram_tensor("x", [n_nodes, P], F16, kind="ExternalInput")
    sel_d = nc.dram_tensor("sel", [P, k_total * P], F16, kind="ExternalInput")
    wt_d = nc.dram_tensor("wt", [P, P], F16, kind="ExternalInput")
    idx_d = nc.dram_tensor("idxp", [P, max_icols], I16, kind="ExternalInput")
    yout_d = nc.dram_tensor("y_out", [n_win * P, P], F32, kind="ExternalOutput")

    G = gather_batch
    SB = sel_batch

    with tile.TileContext(nc) as tc:
        with (
            tc.tile_pool(name="meta", bufs=1) as meta,
            tc.tile_pool(name="gath", bufs=int(os.environ.get("KGBUFS", "8"))) as gpool,
            tc.tile_pool(name="sel", bufs=int(os.environ.get("KSELB", "4"))) as spool,
            tc.tile_pool(name="ytcopy", bufs=2) as ytcopy,
            tc.tile_pool(name="ocopy", bufs=2) as ocopy,
            tc.tile_pool(name="ypsum", bufs=int(os.environ.get("KYPB", "6")), space="PSUM") as ypsum_p,
            tc.tile_pool(name="opsum", bufs=2, space="PSUM") as opsum_p,
        ):
            # --- metadata + constants into SBUF
            wt_sb = meta.tile([P, P], F16)
            nc.sync.dma_start(wt_sb[:], wt_d[:])
            idx_sb = meta.tile([P, max_icols], I16)
            nc.sync.dma_start(idx_sb[:], idx_d[:])

            # per-bank gather state: current batch tile
            gtiles = [None] * n_banks

            def ensure_gather(b, slot):
                g = slot // G
                if gtiles[b] is not None and gtiles[b][0] == g:
                    return gtiles[b][1]
                width = min(G, int(bank_len[b]) - g * G)
                t = gpool.tile([P, G * P], F16, tag="gath")
                c0 = bank_col_base[b]
                nc.gpsimd.dma_gather(
                    out_ap=t[:, : width * P].rearrange("p (k d) -> p k d", d=P),
                    in_ap=x_d[b * bank_size : min((b + 1) * bank_size, n_nodes), :],
                    idxs_ap=idx_sb[
                        :, c0 + g * G * P // 16 : c0 + (g * G + width) * P // 16
                    ],
                    num_idxs=width * P,
                    num_idxs_reg=width * P,
                    elem_size=P,
                    single_packet=single_packet,
                    queue_num=(b % n_queues),
                )
                gtiles[b] = (g, t)
                return t

            # sel stream: tiles of SB chunks loaded by plain DMA
            stiles = [None]

            def ensure_sel(k):
                t = k // SB
                if stiles[0] is not None and stiles[0][0] == t:
                    return stiles[0][1]
                width = min(SB, k_total - t * SB)
                st = spool.tile([P, SB * P], F16, tag="selst")
                nc.sync.dma_start(
                    st[:, : width * P], sel_d[:, t * SB * P : (t * SB + width) * P]
                )
                stiles[0] = (t, st)
                return st

            k = 0
            for w in range(n_win):
                ypsum = ypsum_p.tile([P, P], F32)  # yT: [feat, dest]
                nchunk = int(win_chunks[w])
                for kk in range(nchunk):
                    b = int(chunk_bank[k])
                    slot = int(chunk_slot[k])
                    gt = ensure_gather(b, slot)
                    s = slot % G
                    st = ensure_sel(k)
                    so = (k % SB) * P
                    nc.tensor.matmul(
                        out=ypsum[:],
                        lhsT=gt[:, s * P : (s + 1) * P],
                        rhs=st[:, so : so + P],
                        start=(kk == 0),
                        stop=(kk == nchunk - 1),
                    )
                    k += 1

                # --- apply W: out[dest, o] = sum_d yT[d, dest] * W.T[d, o]
                yt_sb = ytcopy.tile([P, P], F16)
                nc.scalar.copy(yt_sb[:], ypsum[:])
                o_ps = opsum_p.tile([P, P], F32)
                nc.tensor.matmul(
                    out=o_ps[:], lhsT=yt_sb[:], rhs=wt_sb[:], start=True, stop=True
                )
                o_sb = ocopy.tile([P, P], F32)
                nc.scalar.copy(o_sb[:], o_ps[:])
                nc.scalar.dma_start(yout_d[w * P : (w + 1) * P, :], o_sb[:])

    return nc


def kernel(x, edge_row, edge_col, edge_val, W, _trace=False):
    global LAST_RESULTS
    x = np.asarray(x, dtype=np.float32)
    W = np.asarray(W, dtype=np.float32)
    n_nodes = x.shape[0]
    assert x.shape[1] == P and W.shape == (P, P)

    (
        wrap_sb,
        sel,
        chunk_bank,
        chunk_slot,
        bank_len,
        bank_col_base,
        win_chunks,
        n_win,
        shard,
        n_banks,
        bank_size,
        perm,
    ) = _preprocess(edge_row, edge_col, edge_val, n_nodes)
    k_total = len(chunk_bank)
    gather_batch = min(int(os.environ.get("KGATHER", "32")), int(bank_len.max()))
    sel_batch = int(os.environ.get("KSELBATCH", "64"))

    nc = _build(
        n_nodes, k_total, win_chunks, chunk_bank, chunk_slot, bank_len,
        bank_col_base, n_banks, bank_size, gather_batch, sel_batch,
        wrap_sb.shape[2],
    )

    x16 = np.ascontiguousarray(x.astype(np.float16))
    wt = np.ascontiguousarray(W.T.astype(np.float16))

    in_maps = []
    for c in range(NCORES):
        m = {
            "x": x16,
            "sel": np.ascontiguousarray(sel[c]),
            "wt": wt,
        }
        m["idxp"] = np.ascontiguousarray(wrap_sb[c])
        in_maps.append(m)

    if _trace:
        _install_ntff_shim()
    if not nc.is_finalized():
        nc.finalize()
    res = run_bass_kernel_spmd(nc, in_maps, list(range(NCORES)), trace=_trace)
    LAST_RESULTS = res

    out = np.empty((n_nodes, P), dtype=np.float32)
    for c in range(NCORES):
        out[c * shard : (c + 1) * shard] = res.results[c]["y_out"][:shard]
    if perm is not None:
        out = out[perm]
    return out



# revision 10
# speedup vs baseline: 1.3556x; 1.3556x over previous
"""Trainium2 Bass kernel for nn_Aggregate (GNN message passing / COO SpMM + Linear).

Computes: y = segment_sum(edge_val[:,None] * x[edge_col], edge_row, N) @ W.T

Strategy (8 NeuronCores, SPMD):
  - Shard destination nodes across the 8 cores (N/8 rows each); sort edges by
    destination on the host and route each edge to the core owning its dest row.
  - Replicate x (fp16) in every core's HBM. Each core gathers x[edge_col] rows
    for its edges with gpsimd dma_gather (int16 indices -> x is split into row
    banks of <=32k rows; edges are grouped by (dest window, bank) with each
    group padded to a multiple of 128).
  - The one-hot selection matrices sel[e, r] = (r == row_local[e]) * val[e]
    are BUILT ON-DEVICE on the (otherwise idle) Vector engine from a compact
    per-edge (row_local, val) fp16 stream: one fused
    tensor_scalar(is_equal, mult) against a constant iota row per chunk.
    This removes the 54 MB/core host-sel HBM stream (~167 us of DMA).
  - Segment-sum on the TensorEngine, transposed: accumulate
    yT[d, r] += sum_e G[e, d] * sel[e, r] in PSUM (lhsT=G chunk, rhs=sel).
  - Per window: out = yT.T @ W.T via one matmul (lhsT=yT_sbuf, rhs=W.T) -- no
    PE transpose; PSUM<->SBUF copies run on the scalar engine.

Self-contained: numpy + the concourse/bass stack at /opt/trn_rl_repo.
"""

import os
import sys

for _p in ("/opt/trn_rl_repo",):
    if _p not in sys.path and os.path.isdir(_p):
        sys.path.insert(0, _p)

import numpy as np

import concourse.bass as bass
import concourse.mybir as mybir
import concourse.tile as tile
from concourse import bacc
from concourse.bass_utils import run_bass_kernel_spmd

P = 128
NCORES = 8
MAX_BANK = 32000  # int16 index headroom
F32 = mybir.dt.float32
F16 = mybir.dt.float16
I16 = mybir.dt.int16

# Populated by the most recent kernel() call (test harness reads these).
LAST_RESULTS = None


def _install_ntff_shim():
    """The agent image's `antenv` lacks `axon_hooks`; provide it so
    run_bass_kernel_spmd(trace=True) can reach the NTFF profiler."""
    import types

    if "antenv.axon_hooks" in sys.modules:
        return
    try:
        from trn_agent_boot.trn_boot import _ntff_profile_via_ctypes
    except ImportError:
        return
    hook = _ntff_profile_via_ctypes("/opt/axon/libaxon_pjrt.so")
    mod = types.ModuleType("antenv.axon_hooks")
    mod.get_axon_ntff_profile_hook = lambda: hook
    mod.set_axon_ntff_profile_hook = lambda h: None
    sys.modules["antenv.axon_hooks"] = mod
    # the artifact upload wants a remote bucket that is unreachable here
    import concourse.bass_utils as _bu

    _bu.upload_artifacts = lambda tmpdir: f"local:{tmpdir}"


def _balance_permutation(row, col, n_nodes, n_banks, bank_size):
    """Permute dest-node ids so per-(window, bank) edge counts pack just under
    multiples of 128 and stay balanced across cores. Shrinks the shared chunk
    count (pure padding -> pure perf; correctness is permutation-independent).
    Returns perm[n_nodes]: node -> new id."""
    NB = n_banks
    shard = n_nodes // NCORES
    n_win = (shard + P - 1) // P
    last_size = shard - (n_win - 1) * P
    n_full = (n_win - 1) * NCORES
    n_windows = n_win * NCORES

    bank_of = col // bank_size
    deg = np.bincount(row * NB + bank_of, minlength=n_nodes * NB).reshape(
        n_nodes, NB
    ).astype(np.int64)

    caps = np.full(n_windows, P, dtype=np.int64)
    caps[n_full:] = last_size

    lam = len(row) / n_windows / max(NB, 1)
    T = int(max(np.floor(lam / P + 1) * P - 6, 1))

    tall = np.zeros((n_windows, NB), dtype=np.int64)
    size = np.zeros(n_windows, dtype=np.int64)
    assign = np.full(n_nodes, -1, dtype=np.int64)

    order = np.argsort(-deg.sum(1), kind="stable")
    nb_pen = max(NB - 1, 1)
    degs = deg[order]
    FULL = 1 << 40
    for i in range(len(order)):
        d = degs[i]
        over_new = tall[:, :nb_pen] + d[None, :nb_pen] - T
        np.maximum(over_new, 0, out=over_new)
        over_old = tall[:, :nb_pen] - T
        np.maximum(over_old, 0, out=over_old)
        cost = (over_new * over_new - over_old * over_old).sum(1)
        cost += np.where(size >= caps, FULL, 0)
        w = int(np.argmin(cost))
        assign[order[i]] = w
        tall[w] += d
        size[w] += 1

    # group similar windows into grid rows so max-over-cores stays tight
    chunk_demand = (tall + P - 1) // P
    full_ids = np.arange(n_full)
    keys = [chunk_demand[full_ids, b] for b in range(NB)] + [tall[full_ids, NB - 1]]
    lex = np.lexsort(tuple(reversed(keys)))
    full_sorted = full_ids[lex]

    perm = np.empty(n_nodes, dtype=np.int64)
    order_in_win = np.argsort(assign, kind="stable")
    win_start = np.searchsorted(assign[order_in_win], np.arange(n_windows))
    win_end = np.concatenate([win_start[1:], [n_nodes]])

    for r in range(n_win - 1):
        for c in range(NCORES):
            w = int(full_sorted[r * NCORES + c])
            nodes = order_in_win[win_start[w] : win_end[w]]
            base = c * shard + r * P
            perm[nodes] = base + np.arange(len(nodes))
    for c in range(NCORES):
        w = n_full + c
        nodes = order_in_win[win_start[w] : win_end[w]]
        base = c * shard + (n_win - 1) * P
        perm[nodes] = base + np.arange(len(nodes))

    return perm


def _preprocess(edge_row, edge_col, edge_val, n_nodes):
    """Sort edges by dest, shard by dest across cores, group each core's
    window edges by source bank, pad each (window, bank) group to a multiple
    of 128 (shared counts across cores for SPMD).

    Returns:
      wrap_sb: [NCORES, P, max_icols] int16 (dma_gather index wrap, packed)
      rlval: [NCORES, P, 2*k_total] fp32 per-edge metadata for the on-device
           sel build (rlval[c, p, 2k] = local dest row of edge at slot (p, k),
            rlval[c, p, 2k+1] = its edge_val; val 0 for pad slots)
      chunk_bank, chunk_slot: [K_total] int arrays: which bank stream + slot
        each global chunk reads from
      win_chunks: [n_win] number of chunks per window
      n_win, shard, n_banks, bank_size
    """
    shard = n_nodes // NCORES
    n_win = (shard + P - 1) // P
    n_banks = max(1, -(-n_nodes // MAX_BANK))
    bank_size = -(-n_nodes // n_banks)
    NB = n_banks

    row = np.asarray(edge_row).astype(np.int64).ravel()
    col = np.asarray(edge_col).astype(np.int64).ravel()
    val = np.asarray(edge_val).astype(np.float32).ravel()

    if os.environ.get("KBAL", "1") == "1" and n_nodes % NCORES == 0:
        perm = _balance_permutation(row, col, n_nodes, n_banks, bank_size)
        row = perm[row]
    else:
        perm = None

    core_of = row // shard
    win_of = (row - core_of * shard) // P
    row_local = (row - core_of * shard - win_of * P).astype(np.int64)
    bank_of = col // bank_size

    # group key per edge: (core, win, bank)
    key = (core_of * n_win + win_of) * NB + bank_of
    counts = np.bincount(key, minlength=NCORES * n_win * NB).reshape(
        NCORES, n_win, NB
    )
    # shared chunk counts: max over cores
    chunks_wb = (counts.max(axis=0) + P - 1) // P  # [n_win, NB]
    # ensure every window has at least one chunk
    empty = chunks_wb.sum(axis=1) == 0
    chunks_wb[empty, 0] = 1

    win_chunks = chunks_wb.sum(axis=1)  # [n_win]
    k_total = int(win_chunks.sum())

    # global chunk order: w-major, then bank
    flat_chunks = chunks_wb.ravel()  # [(w,b)] -> count
    grp_chunk_base = np.concatenate([[0], np.cumsum(flat_chunks)])[:-1]  # global
    # per-bank slot base for each (w,b) group
    bank_len = chunks_wb.sum(axis=0)  # [NB] chunks per bank stream
    grp_bank_base = np.zeros((n_win, NB), dtype=np.int64)
    grp_bank_base[1:] = np.cumsum(chunks_wb[:-1], axis=0)

    # chunk -> (bank, slot) mapping
    chunk_bank = np.zeros(k_total, dtype=np.int64)
    chunk_slot = np.zeros(k_total, dtype=np.int64)
    for w in range(n_win):
        for b in range(NB):
            n = chunks_wb[w, b]
            if n == 0:
                continue
            g0 = grp_chunk_base[w * NB + b]
            chunk_bank[g0 : g0 + n] = b
            chunk_slot[g0 : g0 + n] = grp_bank_base[w, b] + np.arange(n)

    wrap_idx = [
        np.zeros((NCORES, int(bank_len[b]) * P), dtype=np.int16) for b in range(NB)
    ]

    order = np.argsort(key, kind="stable")
    key_s = key[order]
    # position of each edge within its (core,win,bank) group
    grp_change = np.concatenate([[True], key_s[1:] != key_s[:-1]])
    grp_start = np.flatnonzero(grp_change)
    rep = np.diff(np.concatenate([grp_start, [key_s.shape[0]]]))
    pos_in_grp = np.arange(key_s.shape[0]) - np.repeat(grp_start, rep)

    core_s = key_s // (n_win * NB)
    wb_s = key_s % (n_win * NB)
    w_s = wb_s // NB
    b_s = wb_s % NB
    gchunk = grp_chunk_base[wb_s] + pos_in_grp // P  # global chunk id
    p_s = pos_in_grp % P
    bslot = grp_bank_base[w_s, b_s] + pos_in_grp // P  # bank-stream slot
    q_s = bslot * P + p_s  # bank-stream position

    col_rebased = (col[order] - b_s * bank_size).astype(np.int16)
    rl_s = row_local[order]
    v_s = val[order]

    # compact per-edge metadata for the on-device sel build, fp16:
    # rlval[c, p, 2k] = rl, rlval[c, p, 2k+1] = val (pad slots stay 0)
    rlval = np.zeros((NCORES, P, 2 * k_total), dtype=np.float32)
    rlval[core_s, p_s, 2 * gchunk] = rl_s
    rlval[core_s, p_s, 2 * gchunk + 1] = v_s

    for c in range(NCORES):
        m = core_s == c
        for b in range(NB):
            mb = m & (b_s == b)
            wrap_idx[b][c, q_s[mb]] = col_rebased[mb]

    # packed wrap layout. Two modes:
    #  KQ=1 (default): ONE queue, each bank stream gets its own column range
    #    [bank_col_base[b], ...); idx wrapped in 16 partitions and replicated
    #    to all 128 partitions (8 Q7 cores each read their copy -> full-rate
    #    descriptor generation, benchmark configuration).
    #  KQ=4: legacy 4-queue banding -- bank b in partition band 32b..32b+31,
    #    replicated to the band's two 16-row groups.
    legacy_q = os.environ.get("KQ", "4") == "4"
    if legacy_q:
        max_cols = max(int(bank_len[b]) * P // 16 for b in range(NB))
        bank_col_base = [0] * NB
        wrap_sb = np.zeros((NCORES, P, max_cols), dtype=np.int16)
        for b in range(NB):
            lb = int(bank_len[b]) * P
            if lb == 0:
                continue
            band = (b % 4) * 32
            for c in range(NCORES):
                blk = wrap_idx[b][c].reshape(lb // 16, 16).T  # [16, lb/16]
                wrap_sb[c, band : band + 16, : lb // 16] = blk
                wrap_sb[c, band + 16 : band + 32, : lb // 16] = blk
    else:
        bank_col_base = np.concatenate(
            [[0], np.cumsum([int(bank_len[b]) * P // 16 for b in range(NB)])]
        )[:-1].tolist()
        max_cols = sum(int(bank_len[b]) * P // 16 for b in range(NB))
        wrap_sb = np.zeros((NCORES, P, max_cols), dtype=np.int16)
        for b in range(NB):
            lb = int(bank_len[b]) * P
            if lb == 0:
                continue
            c0 = bank_col_base[b]
            for c in range(NCORES):
                blk = wrap_idx[b][c].reshape(lb // 16, 16).T  # [16, lb/16]
                wrap_sb[c, :, c0 : c0 + lb // 16] = np.tile(blk, (8, 1))

    return (
        wrap_sb,
        rlval,
        chunk_bank,
        chunk_slot,
        bank_len.astype(np.int64),
        bank_col_base,
        win_chunks.astype(np.int64),
        n_win,
        shard,
        NB,
        bank_size,
        perm,
    )


def _build(n_nodes, k_total, win_chunks, chunk_bank, chunk_slot, bank_len,
           bank_col_base, n_banks, bank_size, gather_batch, max_icols):
    """Build the SPMD Bass program (same program on all 8 cores)."""
    n_win = len(win_chunks)
    legacy_q = os.environ.get("KQ", "4") == "4"
    n_queues = min(4, max(1, n_banks)) if legacy_q else 1
    single_packet = os.environ.get("KSP", "0") == "1"
    nc = bacc.Bacc("TRN2", target_bir_lowering=False, debug=False,
                   num_swdge_queues=n_queues,
                   dynamic_dma_scratch_size=int(os.environ.get("KSCRATCH", "32768")))

    x_d = nc.dram_tensor("x", [n_nodes, P], F16, kind="ExternalInput")
    rlval_d = nc.dram_tensor("rlval", [P, 2 * k_total], F32, kind="ExternalInput")
    iota_d = nc.dram_tensor("iota", [P, P], F16, kind="ExternalInput")
    wt_d = nc.dram_tensor("wt", [P, P], F16, kind="ExternalInput")
    idx_d = nc.dram_tensor("idxp", [P, max_icols], I16, kind="ExternalInput")
    yout_d = nc.dram_tensor("y_out", [n_win * P, P], F32, kind="ExternalOutput")

    G = gather_batch
    WB = int(os.environ.get("KWB", "8"))  # windows per output DMA batch

    with tile.TileContext(nc) as tc:
        with (
            tc.tile_pool(name="meta", bufs=1) as meta,
            tc.tile_pool(name="gath", bufs=int(os.environ.get("KGBUFS", "8"))) as gpool,
            tc.tile_pool(name="sel", bufs=int(os.environ.get("KSELB", "8"))) as spool,
            tc.tile_pool(name="ytcopy", bufs=2) as ytcopy,
            tc.tile_pool(name="ocopy", bufs=2) as ocopy,
            tc.tile_pool(name="ypsum", bufs=int(os.environ.get("KYPB", "6")), space="PSUM") as ypsum_p,
            tc.tile_pool(name="opsum", bufs=2, space="PSUM") as opsum_p,
        ):
            # --- metadata + constants into SBUF
            wt_sb = meta.tile([P, P], F16)
            nc.sync.dma_start(wt_sb[:], wt_d[:])
            idx_sb = meta.tile([P, max_icols], I16)
            nc.sync.dma_start(idx_sb[:], idx_d[:])
            rlval_sb = meta.tile([P, 2 * k_total], F32)
            nc.sync.dma_start(rlval_sb[:], rlval_d[:])
            iota_sb = meta.tile([P, P], F16)
            nc.sync.dma_start(iota_sb[:], iota_d[:])

            # per-bank gather state: current batch tile
            gtiles = [None] * n_banks

            def ensure_gather(b, slot):
                g = slot // G
                if gtiles[b] is not None and gtiles[b][0] == g:
                    return gtiles[b][1]
                width = min(G, int(bank_len[b]) - g * G)
                t = gpool.tile([P, G * P], F16, tag="gath")
                c0 = bank_col_base[b]
                nc.gpsimd.dma_gather(
                    out_ap=t[:, : width * P].rearrange("p (k d) -> p k d", d=P),
                    in_ap=x_d[b * bank_size : min((b + 1) * bank_size, n_nodes), :],
                    idxs_ap=idx_sb[
                        :, c0 + g * G * P // 16 : c0 + (g * G + width) * P // 16
                    ],
                    num_idxs=width * P,
                    num_idxs_reg=width * P,
                    elem_size=P,
                    single_packet=single_packet,
                    queue_num=(b % n_queues),
                )
                gtiles[b] = (g, t)
                return t

            k = 0
            o_sb = None
            for w in range(n_win):
                ypsum = ypsum_p.tile([P, P], F32)  # yT: [feat, dest]
                nchunk = int(win_chunks[w])
                for kk in range(nchunk):
                    b = int(chunk_bank[k])
                    slot = int(chunk_slot[k])
                    gt = ensure_gather(b, slot)
                    s = slot % G
                    # sel[e, r] = (iota[r] == rl[e]) * val[e], on the DVE
                    st = spool.tile([P, P], F16, tag="selst")
                    nc.vector.tensor_scalar(
                        st[:],
                        iota_sb[:],
                        rlval_sb[:, 2 * k : 2 * k + 1],
                        rlval_sb[:, 2 * k + 1 : 2 * k + 2],
                        mybir.AluOpType.is_equal,
                        mybir.AluOpType.mult,
                    )
                    nc.tensor.matmul(
                        out=ypsum[:],
                        lhsT=gt[:, s * P : (s + 1) * P],
                        rhs=st[:],
                        start=(kk == 0),
                        stop=(kk == nchunk - 1),
                    )
                    k += 1

                # --- apply W: out[dest, o] = sum_d yT[d, dest] * W.T[d, o]
                yt_sb = ytcopy.tile([P, P], F16)
                nc.scalar.copy(yt_sb[:], ypsum[:])
                o_ps = opsum_p.tile([P, P], F32)
                nc.tensor.matmul(
                    out=o_ps[:], lhsT=yt_sb[:], rhs=wt_sb[:], start=True, stop=True
                )
                if w % WB == 0:
                    o_sb = ocopy.tile([P, WB * P], F32, tag="ocopy")
                j = w % WB
                nc.scalar.copy(o_sb[:, j * P : (j + 1) * P], o_ps[:])
                if j == WB - 1 or w == n_win - 1:
                    w0 = w - j
                    nc.scalar.dma_start(
                        yout_d[w0 * P : (w + 1) * P, :].rearrange(
                            "(b p) d -> p b d", p=P
                        ),
                        o_sb[:, : (j + 1) * P].rearrange("p (b d) -> p b d", d=P),
                    )

    return nc


def kernel(x, edge_row, edge_col, edge_val, W, _trace=False):
    global LAST_RESULTS
    x = np.asarray(x, dtype=np.float32)
    W = np.asarray(W, dtype=np.float32)
    n_nodes = x.shape[0]
    assert x.shape[1] == P and W.shape == (P, P)

    (
        wrap_sb,
        rlval,
        chunk_bank,
        chunk_slot,
        bank_len,
        bank_col_base,
        win_chunks,
        n_win,
        shard,
        n_banks,
        bank_size,
        perm,
    ) = _preprocess(edge_row, edge_col, edge_val, n_nodes)
    k_total = len(chunk_bank)
    gather_batch = min(int(os.environ.get("KGATHER", "32")), int(bank_len.max()))

    nc = _build(
        n_nodes, k_total, win_chunks, chunk_bank, chunk_slot, bank_len,
        bank_col_base, n_banks, bank_size, gather_batch,
        wrap_sb.shape[2],
    )

    x16 = np.ascontiguousarray(x.astype(np.float16))
    wt = np.ascontiguousarray(W.T.astype(np.float16))
    iota = np.broadcast_to(
        np.arange(P, dtype=np.float16)[None, :], (P, P)
    ).copy()

    in_maps = []
    for c in range(NCORES):
        m = {
            "x": x16,
            "rlval": np.ascontiguousarray(rlval[c]),
            "iota": iota,
            "wt": wt,
        }
        m["idxp"] = np.ascontiguousarray(wrap_sb[c])
        in_maps.append(m)

    if _trace:
        _install_ntff_shim()
    if not nc.is_finalized():
        nc.finalize()
    res = run_bass_kernel_spmd(nc, in_maps, list(range(NCORES)), trace=_trace)
    LAST_RESULTS = res

    out = np.empty((n_nodes, P), dtype=np.float32)
    for c in range(NCORES):
        out[c * shard : (c + 1) * shard] = res.results[c]["y_out"][:shard]
    if perm is not None:
        out = out[perm]
    return out

